# revision 73
# baseline (speedup 1.0000x reference)
"""Trainium2 Bass kernel for AudioAttentionMapGenerator.

Math (reference):
    sigma = exp(log_sigma); c = 0.5 / (sigma^2 + 1e-6)
    w_n   = attn_weights * mask
    map[b,h,w] = sum_n w_n * exp(-c*((h-v_bn)^2 + (w-u_bn)^2))
    out = map / (max_hw(map) + 1e-6)

The Gaussian is separable: per sample  map = Gy^T @ (w * Gx)  — two
(112,N)@(N,W) matmuls.  Per gaussian axis the exponent is expanded as
    -c (x-u)^2 = cneg*(x^2 - 2 u x) + cneg*u^2
so each gaussian row block is ONE DVE scalar_tensor_tensor
(d = grid*(-2u) + grid2; grid is generated on-chip by a prefix scan, so no
DMA sits on the critical path) followed by ONE activation
Exp(scale=cneg, bias=cneg*u^2).

Sharding: data-parallel over B=16 across 8 cores (2 samples/core), N=128 on
SBUF partitions, H split 112+112.  Matmuls run as float32r with the moving
free dim padded to 256 (full rate per the PE cost model); a burst of tiny
dependency-free matmuls beforehand keeps the PE array in its continuous-busy
ramp so the real matmuls hit the warm clock.  Per-sample max: one fused
free-dim reduce (DVE) + partition all-reduce (GPSIMD); both normalization
scales run on ACT (PSUM-adjacent, avoids cross-engine same-PSUM-tensor
serialization) into per-chunk staging tiles, each shipped by its own DMA as
soon as it is ready (4 DMAs, descriptors pipelined).
"""

import sys

import numpy as np

if "/opt/trn_rl_repo" not in sys.path:
    sys.path.insert(0, "/opt/trn_rl_repo")

B, N, H, W = 16, 128, 224, 224
NCORES = 8
BPC = B // NCORES  # samples per core
HC = H // 2  # 112 — H chunk (stationary free-dim <= 128)
WP = 256  # moving operand padded width (float32r full-rate needs >=256)

_CACHE = {}


def _build():
    if "nc" in _CACHE:
        return _CACHE["nc"]

    from contextlib import ExitStack

    import concourse.bass_isa as bass_isa
    import concourse.tile as tile
    from concourse import bacc, mybir

    f32 = mybir.dt.float32
    f32r = mybir.dt.float32r
    AF = mybir.ActivationFunctionType
    AX = mybir.AxisListType
    OP = mybir.AluOpType

    nc = bacc.Bacc(
        "TRN2",
        target_bir_lowering=False,
        debug=False,
        enable_asserts=False,
        num_devices=NCORES,
    )
    # packed per-core input: [u0,u1,v0,v1, aw0,aw1, m0,m1, log_sigma, pad...]
    pk = nc.dram_tensor("pk", (N, 12), f32, kind="ExternalInput").ap()
    out = nc.dram_tensor("out", (BPC, H, W), f32, kind="ExternalOutput").ap()

    from concourse.tile import add_dep_helper

    with ExitStack() as ctx:
        tc = ctx.enter_context(tile.TileContext(nc))
        consts = ctx.enter_context(tc.tile_pool(name="consts", bufs=1))
        work = ctx.enter_context(tc.tile_pool(name="work", bufs=2))
        small = ctx.enter_context(tc.tile_pool(name="small", bufs=4))
        psum = ctx.enter_context(tc.tile_pool(name="psum", bufs=2, space="PSUM"))

        # ---- constants / per-core inputs ----
        pkt = consts.tile([128, 12], f32)
        nc.sync.dma_start(out=pkt, in_=pk)
        # grid = [0..W) per partition, generated on-chip (prefix scan of ones)
        # so no DMA sits on the critical path; grid2 = grid^2
        ones = consts.tile([128, W], f32)
        nc.vector.memset(ones, 1.0)
        grid = consts.tile([128, W], f32)
        nc.vector.tensor_tensor_scan(grid, ones, ones, -1.0, OP.add, OP.mult)
        grid2 = consts.tile([128, W], f32)
        nc.vector.tensor_mul(grid2, grid, grid)

        # cneg = -0.5 / (exp(2*log_sigma) + 1e-6), replicated on all partitions:
        # reciprocal((sig2 + 1e-6) * -2)
        sig2 = consts.tile([128, 1], f32)
        nc.scalar.activation(sig2, pkt[:, 8:9], AF.Exp, scale=2.0)
        sig2e = consts.tile([128, 1], f32)
        nc.vector.tensor_scalar(sig2e, sig2, 1e-6, -2.0, OP.add, OP.mult)
        cneg = consts.tile([128, 1], f32)
        i_cneg = nc.vector.reciprocal(cneg, sig2e)

        # weights = attn * mask
        wt = consts.tile([128, BPC], f32)
        nc.vector.tensor_mul(wt, pkt[:, 4:6], pkt[:, 6:8])

        # n2 = -2*[u0,u1,v0,v1];  ncrd = -coords;  bias4 = cneg*coord^2
        n2 = consts.tile([128, 4], f32)
        nc.vector.tensor_scalar_mul(n2, pkt[:, 0:4], -2.0)
        ncrd = consts.tile([128, 4], f32)
        nc.vector.tensor_scalar_mul(ncrd, pkt[:, 0:4], -1.0)
        pc2 = consts.tile([128, 4], f32)
        nc.vector.tensor_mul(pc2, pkt[:, 0:4], pkt[:, 0:4])
        bias4 = consts.tile([128, 4], f32)
        i_bias4 = nc.vector.tensor_scalar(bias4, pc2, cneg[:, 0:1], None, OP.mult)

        # PE warm-up: small dependency-free matmuls keep the PE array in its
        # continuous-busy ramp so the real matmuls run at full rate
        pwarm = psum.tile([1, 64], f32, tag="pwarm")
        for _ in range(14):
            nc.tensor.matmul(pwarm, ones[0:1, 0:1], ones[0:1, 0:64])

        # ---- per-sample pipeline ----
        # two phases: gaussians+matmuls for BOTH samples first, then the
        # normalization chains. Program order breaks scheduler ties, so both
        # samples' tiny weight-multiplies beat the fat reduces to the DVE queue.
        pmaps = []
        for b in range(BPC):
            # x side first; weight applied on the way to f32r (DVE) while the
            # y-side Exp runs on ACT
            ddx = work.tile([128, W], f32, tag="ddx")
            i_sx = nc.vector.scalar_tensor_tensor(
                ddx, grid, n2[:, b : b + 1], grid2, OP.mult, OP.add
            )
            gx = work.tile([128, W], f32, tag="gx")
            nc.scalar.activation(
                gx, ddx, AF.Exp, scale=cneg[:, 0:1], bias=bias4[:, b : b + 1]
            )
            wgx = work.tile([128, WP], f32r, tag="wgx")
            nc.vector.memset(wgx[:, W:WP].bitcast(mybir.dt.uint32), 0)
            nc.vector.tensor_scalar_mul(wgx[:, 0:W], gx, wt[:, b : b + 1])
            # y side exponent on DVE (one STT), Exp on ACT
            ddy = work.tile([128, W], f32, tag="ddy")
            i_sy = nc.vector.scalar_tensor_tensor(
                ddy, grid, n2[:, 2 + b : 3 + b], grid2, OP.mult, OP.add
            )
            gy = work.tile([128, W], f32r, tag="gy")
            nc.scalar.activation(
                gy, ddy, AF.Exp, scale=cneg[:, 0:1], bias=bias4[:, 2 + b : 3 + b]
            )
            # keep the tiny cneg/bias4 chain ahead of the fat STTs in the
            # in-order DVE queue
            for i_stt in (i_sx, i_sy):
                add_dep_helper(i_stt.ins, i_cneg.ins, sync=False, reason="cneg first")
                add_dep_helper(i_stt.ins, i_bias4.ins, sync=False, reason="bias4 first")

            pmap = psum.tile([HC, 2, WP], f32, tag="pmap")
            nc.tensor.matmul(pmap[:, 0, :], gy[:, 0:HC], wgx)
            nc.tensor.matmul(pmap[:, 1, :], gy[:, HC:H], wgx)
            pmaps.append(pmap)

        for b in range(BPC):
            pmap = pmaps[b]
            # per-sample max over the whole map: one fused free-dim reduce
            # (DVE) + partition all-reduce (GPSIMD)
            mrow = small.tile([HC, 1], f32, tag="mrow")
            nc.vector.reduce_max(mrow, pmap[:, :, 0:W], axis=AX.XY)
            mall = small.tile([HC, 1], f32, tag="mall")
            nc.gpsimd.partition_all_reduce(
                mall, mrow, channels=HC, reduce_op=bass_isa.ReduceOp.max
            )
            mxe = small.tile([HC, 1], f32, tag="mxe")
            nc.vector.tensor_scalar_add(mxe, mall, 1e-6)
            rs = small.tile([HC, 1], f32, tag="rs")
            nc.vector.reciprocal(rs, mxe)

            o0 = work.tile([HC, W], f32, tag="o0")
            nc.scalar.mul(o0, pmap[:, 0, 0:W], rs[:, 0:1])
            o1 = work.tile([HC, W], f32, tag="o1")
            nc.scalar.mul(o1, pmap[:, 1, 0:W], rs[:, 0:1])
            nc.sync.dma_start(out=out[b, 0:HC, :], in_=o0)
            nc.sync.dma_start(out=out[b, HC:H, :], in_=o1)

    nc.compile()
    _CACHE["nc"] = nc
    return nc


def kernel(pixel_coords, attn_weights, in_frame_mask, log_sigma, **kwargs):
    pixel_coords = np.asarray(pixel_coords, dtype=np.float32)
    attn_weights = np.asarray(attn_weights, dtype=np.float32)
    mask_f = np.asarray(in_frame_mask).astype(np.float32)
    ls = float(np.asarray(log_sigma, dtype=np.float32))

    nc = _build()
    from concourse.bass_utils import run_bass_kernel_spmd

    in_maps = []
    for i in range(NCORES):
        sl = slice(i * BPC, (i + 1) * BPC)
        pc = pixel_coords[sl]  # (BPC, N, 2)
        aw = attn_weights[sl]  # (BPC, N)
        mf = mask_f[sl]
        pkt = np.zeros((N, 12), dtype=np.float32)
        pkt[:, 0] = pc[0, :, 0]
        pkt[:, 1] = pc[1, :, 0]
        pkt[:, 2] = pc[0, :, 1]
        pkt[:, 3] = pc[1, :, 1]
        pkt[:, 4] = aw[0]
        pkt[:, 5] = aw[1]
        pkt[:, 6] = mf[0]
        pkt[:, 7] = mf[1]
        pkt[:, 8] = ls
        in_maps.append({"pk": pkt})
    res = run_bass_kernel_spmd(nc, in_maps, core_ids=list(range(NCORES)))
    return np.concatenate([r["out"] for r in res.results], axis=0)
